# revision 18
# baseline (speedup 1.0000x reference)
"""Trainium2 Bass kernel for nn_AdaptivePoolingClassifier.

Math: the reference MLP has no nonlinearity between its first three layers,
so they collapse into one 128x128 matmul:
    h3 = x @ Wc + bc          with Wc = W1@W2@W3, bc = ((b1@W2+b2)@W3+b3)
    p  = relu(h3) @ W4 + b4                       # [N, 5]
    q  = alpha * p = relu(h3) @ (W4*alpha) + b4*alpha
    out[n] = sum_r p[r,n]*e^{q[r,n]} / sum_r e^{q[r,n]}   # softmax pooling

Sharding: rows split across 8 NeuronCores. Each core returns partial sums
(sum e^q and sum p*e^q, kept per-partition/per-group to stay cheap on-chip);
the host adds the 8 partials and does the final 5-element division.

Per-core dataflow (bf16 compute, f32 accumulation):
  cast-DMA (f32->bf16, SWDGE)  ->  x_sb [128 rows, 512]
  PE transpose-mode per 128-row block -> xT (bf16, PSUM) -> DVE evac to SBUF
  PE: h3T = Wc^T @ xT  (one N=512 matmul)
  ACT/DVE: relu(h3T + bc) -> SBUF (bias is per-partition here)
  PE per 128-row block: pq[rows, 10] = relu_blk^T-stationary @ W45, PSUM-
     accumulated 32 blocks per bank, + one K=1 bias matmul for b45
  ACT: e = exp(q)   DVE: pe = p*e ; acc_e += e ; acc_pe += pe
  DMA out: [128, 320] f32 partials per core.
"""

import sys
import types
import numpy as np

_REPO = "/opt/trn_rl_repo"
if _REPO not in sys.path:
    sys.path.insert(0, _REPO)

import concourse.bacc as bacc  # noqa: E402
import concourse.bass as bass  # noqa: E402
import concourse.mybir as mybir  # noqa: E402
from concourse import tile  # noqa: E402
from concourse.bass_utils import run_bass_kernel_spmd  # noqa: E402

import ml_dtypes  # noqa: E402

BF16 = ml_dtypes.bfloat16

N_CORES = 8
D = 128
NQ = 10  # p (5) | q (5)
TILE_ROWS = 1024
BLOCKS_PER_TILE = TILE_ROWS // 128  # 8
CHUNK_TILES = 4  # stats chunk = 4096 rows
GROUPS_PER_CHUNK = CHUNK_TILES * BLOCKS_PER_TILE  # 32
STATS_W = GROUPS_PER_CHUNK * 5  # 160
ACT_COLS = 1024  # columns of the relu evac done on ScalarE (rest on DVE)


def build_kernel(rows_per_core: int, act_cols: int = ACT_COLS):
    """Build the per-core Bacc graph. rows_per_core must divide into chunks."""
    assert rows_per_core % (TILE_ROWS * CHUNK_TILES) == 0
    n_chunks = rows_per_core // (TILE_ROWS * CHUNK_TILES)
    n_tiles = rows_per_core // TILE_ROWS

    f32 = mybir.dt.float32
    bf16 = mybir.dt.bfloat16

    nc = bacc.Bacc("TRN2", target_bir_lowering=False, debug=False,
                   num_devices=N_CORES)

    x_ext = nc.declare_dram_parameter("x", [rows_per_core, D], f32,
                                      isOutput=False)
    # packed bf16 consts: [wc | ident | w45 | ones1-col | b45t rows]
    # layout [128, 128+128+10+1+3] -> b45t is stored as 3 columns of
    # 107 rows... simpler: separate region appended as extra columns on
    # partition 0 is awkward; instead pack b45t as columns 267:270 won't
    # fit 320 values. Keep b45t in its own small parameter.
    cb_ext = nc.declare_dram_parameter("cb", [D, 3 * D + NQ], bf16,
                                       isOutput=False)
    bc_ext = nc.declare_dram_parameter("bc", [D, 1], f32, isOutput=False)
    b45t_ext = nc.declare_dram_parameter("b45t", [1, GROUPS_PER_CHUNK * NQ],
                                         bf16, isOutput=False)
    out_ext = nc.declare_dram_parameter("out", [D, 2 * STATS_W], f32,
                                        isOutput=True)

    # DMA granule: partition p holds 2*BLOCKS_PER_TILE *consecutive* rows
    # (8 KB contiguous per partition -> efficient DMA descriptors); one DMA
    # feeds two compute tiles. Row order within a tile is permuted vs. DRAM,
    # which is fine: the softmax pooling is row-permutation invariant.
    DMA_BLOCKS = 2 * BLOCKS_PER_TILE
    x_r = x_ext.ap().rearrange("(g p k) f -> g p k f", p=128, k=DMA_BLOCKS)

    with tile.TileContext(nc) as tc:
        with (
            tc.tile_pool(name="consts", bufs=1) as cpool,
            tc.tile_pool(name="xin", bufs=6) as xpool,
            tc.tile_pool(name="xt", bufs=4) as xtpool,
            tc.tile_pool(name="relu", bufs=4) as rpool,
            tc.tile_pool(name="stats", bufs=3) as spool,
            tc.tile_pool(name="acc", bufs=1) as apool,
            tc.tile_pool(name="ps_xt", bufs=2, space="PSUM") as ps_xt,
            tc.tile_pool(name="ps_h3", bufs=2, space="PSUM") as ps_h3,
            tc.tile_pool(name="ps_pq", bufs=2, space="PSUM") as ps_pq,
        ):
            cb_sb = cpool.tile([D, 3 * D + NQ], bf16)
            nc.sync.dma_start(out=cb_sb[:], in_=cb_ext[:])
            wc_sb = cb_sb[:, 0:D]
            ident_sb = cb_sb[:, D:2 * D]
            w45_sb = cb_sb[:, 2 * D:2 * D + NQ]
            ones1_sb = cb_sb[0:1, 2 * D + NQ:3 * D + NQ]  # row 0 = ones
            bc_sb = cpool.tile([D, 1], f32)
            nc.scalar.dma_start(out=bc_sb[:], in_=bc_ext[:])
            b45t_sb = cpool.tile([1, GROUPS_PER_CHUNK * NQ], bf16)
            nc.scalar.dma_start(out=b45t_sb[:], in_=b45t_ext[:])

            acc_e = apool.tile([D, STATS_W], f32)
            acc_pe = apool.tile([D, STATS_W], f32)

            x_dma = None
            for chunk in range(n_chunks):
                pq = ps_pq.tile([D, GROUPS_PER_CHUNK * NQ], f32)
                for t in range(CHUNK_TILES):
                    g_tile = chunk * CHUNK_TILES + t
                    half = g_tile % 2
                    if half == 0:
                        x_dma = xpool.tile([D, 2 * TILE_ROWS], bf16)
                        # SWDGE cast DMA: f32 HBM -> bf16 SBUF
                        nc.gpsimd.dma_start(out=x_dma[:],
                                            in_=x_r[g_tile // 2])
                    x_sb = x_dma[:, half * TILE_ROWS:(half + 1) * TILE_ROWS]

                    xt_ps = ps_xt.tile([D, TILE_ROWS], bf16)
                    for k in range(BLOCKS_PER_TILE):
                        nc.tensor.transpose(
                            xt_ps[:, 128 * k:128 * (k + 1)],
                            x_sb[:, 128 * k:128 * (k + 1)],
                            ident_sb[:],
                        )
                    xt_sb = xtpool.tile([D, TILE_ROWS], bf16)
                    nc.vector.tensor_copy(xt_sb[:], xt_ps[:])

                    h3_ps = ps_h3.tile([D, TILE_ROWS], f32)
                    for half in range(TILE_ROWS // 512):
                        nc.tensor.matmul(
                            h3_ps[:, 512 * half:512 * (half + 1)],
                            wc_sb[:],
                            xt_sb[:, 512 * half:512 * (half + 1)],
                            start=True, stop=True)

                    relu_sb = rpool.tile([D, TILE_ROWS], bf16)
                    a = min(act_cols, TILE_ROWS)
                    if a > 0:
                        nc.scalar.activation(
                            relu_sb[:, 0:a], h3_ps[:, 0:a],
                            mybir.ActivationFunctionType.Relu,
                            bias=bc_sb[:, 0:1], scale=1.0,
                        )
                    if a < TILE_ROWS:
                        nc.vector.tensor_scalar(
                            relu_sb[:, a:TILE_ROWS], h3_ps[:, a:TILE_ROWS],
                            bc_sb[:, 0:1], 0.0,
                            mybir.AluOpType.add, mybir.AluOpType.max,
                        )

                    for k in range(BLOCKS_PER_TILE):
                        g = t * BLOCKS_PER_TILE + k
                        nc.tensor.matmul(
                            pq[:, NQ * g:NQ * (g + 1)],
                            relu_sb[:, 128 * k:128 * (k + 1)],
                            w45_sb[:],
                            start=(g == 0), stop=False,
                        )
                # accumulate biases b45 over the whole chunk's pq
                nc.tensor.matmul(pq[:], ones1_sb[0:1, :], b45t_sb[0:1, :],
                                 start=False, stop=True)

                pq3 = pq[:].rearrange("p (g n) -> p g n", n=NQ)
                e_sb = spool.tile([D, STATS_W], f32, tag="e")
                e3 = e_sb[:].rearrange("p (g n) -> p g n", n=5)
                nc.scalar.activation(e3, pq3[:, :, 5:10],
                                     mybir.ActivationFunctionType.Exp)
                pe_sb = spool.tile([D, STATS_W], f32, tag="pe")
                pe3 = pe_sb[:].rearrange("p (g n) -> p g n", n=5)
                nc.vector.tensor_mul(pe3, pq3[:, :, 0:5], e3)
                if chunk == 0:
                    nc.vector.tensor_copy(acc_e[:], e_sb[:])
                    nc.vector.tensor_copy(acc_pe[:], pe_sb[:])
                else:
                    nc.vector.tensor_add(acc_e[:], acc_e[:], e_sb[:])
                    nc.vector.tensor_add(acc_pe[:], acc_pe[:], pe_sb[:])

            nc.sync.dma_start(out=out_ext[:, 0:STATS_W], in_=acc_e[:])
            nc.sync.dma_start(out=out_ext[:, STATS_W:2 * STATS_W],
                              in_=acc_pe[:])

    nc.compile()
    return nc


def _prep_consts(W1, b1, W2, b2, W3, b3, W4, b4, alpha):
    Wc = (W1.astype(np.float64) @ W2.astype(np.float64)
          @ W3.astype(np.float64))
    bc = ((b1.astype(np.float64) @ W2.astype(np.float64)
           + b2.astype(np.float64)) @ W3.astype(np.float64)
          + b3.astype(np.float64))
    W4a = W4.astype(np.float64) * alpha.astype(np.float64)[None, :]
    b4a = b4.astype(np.float64) * alpha.astype(np.float64)
    W45 = np.concatenate([W4.astype(np.float64), W4a], axis=1)  # [128, 10]
    b45 = np.concatenate([b4.astype(np.float64), b4a])  # [10]
    b45t = np.tile(b45, GROUPS_PER_CHUNK)[None, :]  # [1, 320]
    return (
        Wc.astype(BF16),
        bc.astype(np.float32).reshape(D, 1),
        W45.astype(BF16),
        b45t.astype(BF16),
    )


_CACHE = {}


def _get_nc(rows_per_core):
    key = rows_per_core
    if key not in _CACHE:
        _CACHE[key] = build_kernel(rows_per_core)
    return _CACHE[key]


def make_in_maps(x, W1, b1, W2, b2, W3, b3, W4, b4, alpha):
    x = np.asarray(x)
    n_total = x.shape[1]
    rows_per_core = n_total // N_CORES
    wc_bf, bc_f32, w45_bf, b45t_bf = _prep_consts(
        np.asarray(W1), np.asarray(b1), np.asarray(W2), np.asarray(b2),
        np.asarray(W3), np.asarray(b3), np.asarray(W4), np.asarray(b4),
        np.asarray(alpha))
    # packed bf16 const block: [wc | identity | w45 | ones(row 0)]
    cb = np.zeros((D, 3 * D + NQ), dtype=BF16)
    cb[:, 0:D] = wc_bf
    cb[:, D:2 * D] = np.eye(D, dtype=BF16)
    cb[:, 2 * D:2 * D + NQ] = w45_bf
    cb[0, 2 * D + NQ:3 * D + NQ] = np.ones(D, dtype=BF16)

    xs = np.ascontiguousarray(x.reshape(n_total, D))
    in_maps = []
    for c in range(N_CORES):
        in_maps.append({
            "x": xs[c * rows_per_core:(c + 1) * rows_per_core],
            "cb": cb,
            "bc": bc_f32,
            "b45t": b45t_bf,
        })
    return in_maps, rows_per_core


def run(inputs, trace=False, **run_kwargs):
    """Run the kernel; returns (full_output, BassKernelResults)."""
    in_maps, rows_per_core = make_in_maps(**inputs)
    nc = _get_nc(rows_per_core)
    res = run_bass_kernel_spmd(nc, in_maps, list(range(N_CORES)),
                               trace=trace, **run_kwargs)
    return _finish(res.results), res


def kernel(x, W1, b1, W2, b2, W3, b3, W4, b4, alpha):
    out, _ = run(dict(x=x, W1=W1, b1=b1, W2=W2, b2=b2, W3=W3, b3=b3,
                      W4=W4, b4=b4, alpha=alpha))
    return out


def _finish(results):
    S = np.zeros((D, 2 * STATS_W), dtype=np.float64)
    for r in results:
        S += r["out"].astype(np.float64)
    se = S[:, :STATS_W].reshape(D, GROUPS_PER_CHUNK, 5).sum(axis=(0, 1))
    spe = S[:, STATS_W:].reshape(D, GROUPS_PER_CHUNK, 5).sum(axis=(0, 1))
    return (spe / se)[None, :].astype(np.float32)


# revision 21
# speedup vs baseline: 1.0315x; 1.0315x over previous
"""Trainium2 Bass kernel for nn_AdaptivePoolingClassifier.

Math: the reference MLP has no nonlinearity between its first three layers,
so they collapse into one 128x128 matmul:
    h3 = x @ Wc + bc          with Wc = W1@W2@W3, bc = ((b1@W2+b2)@W3+b3)
    p  = relu(h3) @ W4 + b4                       # [N, 5]
    q  = alpha * p = relu(h3) @ (W4*alpha) + b4*alpha
    out[n] = sum_r p[r,n]*e^{q[r,n]} / sum_r e^{q[r,n]}   # softmax pooling

Sharding: rows split across 8 NeuronCores. Each core returns partial sums
(sum e^q and sum p*e^q, kept per-partition/per-group to stay cheap on-chip);
the host adds the 8 partials and does the final 5-element division.

Per-core dataflow (bf16 compute, f32 accumulation):
  cast-DMA (f32->bf16, SWDGE)  ->  x_sb [128 rows, 512]
  PE transpose-mode per 128-row block -> xT (bf16, PSUM) -> DVE evac to SBUF
  PE: h3T = Wc^T @ xT  (one N=512 matmul)
  ACT/DVE: relu(h3T + bc) -> SBUF (bias is per-partition here)
  PE per 128-row block: pq[rows, 10] = relu_blk^T-stationary @ W45, PSUM-
     accumulated 32 blocks per bank, + one K=1 bias matmul for b45
  ACT: e = exp(q)   DVE: pe = p*e ; acc_e += e ; acc_pe += pe
  DMA out: [128, 320] f32 partials per core.
"""

import sys
import types
import numpy as np

_REPO = "/opt/trn_rl_repo"
if _REPO not in sys.path:
    sys.path.insert(0, _REPO)

import concourse.bacc as bacc  # noqa: E402
import concourse.bass as bass  # noqa: E402
import concourse.mybir as mybir  # noqa: E402
from concourse import tile  # noqa: E402
from concourse.bass_utils import run_bass_kernel_spmd  # noqa: E402

import ml_dtypes  # noqa: E402

BF16 = ml_dtypes.bfloat16

N_CORES = 8
D = 128
NQ = 10  # p (5) | q (5)
TILE_ROWS = 1024
BLOCKS_PER_TILE = TILE_ROWS // 128  # 8
CHUNK_TILES = 4  # stats chunk = 4096 rows
GROUPS_PER_CHUNK = CHUNK_TILES * BLOCKS_PER_TILE  # 32
STATS_W = GROUPS_PER_CHUNK * 5  # 160
ACT_COLS = 1024  # columns of the relu evac done on ScalarE (rest on DVE)


def build_kernel(rows_per_core: int, act_cols: int = ACT_COLS):
    """Build the per-core Bacc graph. rows_per_core must divide into chunks."""
    assert rows_per_core % (TILE_ROWS * CHUNK_TILES) == 0
    n_chunks = rows_per_core // (TILE_ROWS * CHUNK_TILES)
    n_tiles = rows_per_core // TILE_ROWS

    f32 = mybir.dt.float32
    bf16 = mybir.dt.bfloat16

    nc = bacc.Bacc("TRN2", target_bir_lowering=False, debug=False,
                   num_devices=N_CORES)

    x_ext = nc.declare_dram_parameter("x", [rows_per_core, D], f32,
                                      isOutput=False)
    # packed bf16 consts: [wc | ident | w45 | ones1-col | b45t rows]
    # layout [128, 128+128+10+1+3] -> b45t is stored as 3 columns of
    # 107 rows... simpler: separate region appended as extra columns on
    # partition 0 is awkward; instead pack b45t as columns 267:270 won't
    # fit 320 values. Keep b45t in its own small parameter.
    cb_ext = nc.declare_dram_parameter("cb", [D, 3 * D + NQ], bf16,
                                       isOutput=False)
    bc_ext = nc.declare_dram_parameter("bc", [D, 1], f32, isOutput=False)
    b45t_ext = nc.declare_dram_parameter("b45t", [1, GROUPS_PER_CHUNK * NQ],
                                         bf16, isOutput=False)
    out_ext = nc.declare_dram_parameter("out", [D, 2 * STATS_W], f32,
                                        isOutput=True)

    # DMA granule: partition p holds 2*BLOCKS_PER_TILE *consecutive* rows
    # (8 KB contiguous per partition -> efficient DMA descriptors); one DMA
    # feeds two compute tiles. Row order within a tile is permuted vs. DRAM,
    # which is fine: the softmax pooling is row-permutation invariant.
    TILES_PER_DMA = 4
    DMA_BLOCKS = TILES_PER_DMA * BLOCKS_PER_TILE
    x_r = x_ext.ap().rearrange("(g p k) f -> g p k f", p=128, k=DMA_BLOCKS)

    with tile.TileContext(nc) as tc:
        with (
            tc.tile_pool(name="consts", bufs=1) as cpool,
            tc.tile_pool(name="xin", bufs=3) as xpool,
            tc.tile_pool(name="xt", bufs=4) as xtpool,
            tc.tile_pool(name="relu", bufs=4) as rpool,
            tc.tile_pool(name="stats", bufs=3) as spool,
            tc.tile_pool(name="acc", bufs=1) as apool,
            tc.tile_pool(name="ps_xt", bufs=2, space="PSUM") as ps_xt,
            tc.tile_pool(name="ps_h3", bufs=2, space="PSUM") as ps_h3,
            tc.tile_pool(name="ps_pq", bufs=2, space="PSUM") as ps_pq,
        ):
            cb_sb = cpool.tile([D, 3 * D + NQ], bf16)
            nc.sync.dma_start(out=cb_sb[:], in_=cb_ext[:])
            wc_sb = cb_sb[:, 0:D]
            ident_sb = cb_sb[:, D:2 * D]
            w45_sb = cb_sb[:, 2 * D:2 * D + NQ]
            ones1_sb = cb_sb[0:1, 2 * D + NQ:3 * D + NQ]  # row 0 = ones
            bc_sb = cpool.tile([D, 1], f32)
            nc.scalar.dma_start(out=bc_sb[:], in_=bc_ext[:])
            b45t_sb = cpool.tile([1, GROUPS_PER_CHUNK * NQ], bf16)
            nc.scalar.dma_start(out=b45t_sb[:], in_=b45t_ext[:])

            acc_e = apool.tile([D, STATS_W], f32)
            acc_pe = apool.tile([D, STATS_W], f32)

            x_dma = None
            for chunk in range(n_chunks):
                pq = ps_pq.tile([D, GROUPS_PER_CHUNK * NQ], f32)
                for t in range(CHUNK_TILES):
                    g_tile = chunk * CHUNK_TILES + t
                    sub = g_tile % TILES_PER_DMA
                    if sub == 0:
                        x_dma = xpool.tile([D, TILES_PER_DMA * TILE_ROWS],
                                           bf16)
                        # SWDGE cast DMA: f32 HBM -> bf16 SBUF
                        nc.gpsimd.dma_start(out=x_dma[:],
                                            in_=x_r[g_tile // TILES_PER_DMA])
                    x_sb = x_dma[:, sub * TILE_ROWS:(sub + 1) * TILE_ROWS]

                    xt_ps = ps_xt.tile([D, TILE_ROWS], bf16)
                    for k in range(BLOCKS_PER_TILE):
                        nc.tensor.transpose(
                            xt_ps[:, 128 * k:128 * (k + 1)],
                            x_sb[:, 128 * k:128 * (k + 1)],
                            ident_sb[:],
                        )
                    xt_sb = xtpool.tile([D, TILE_ROWS], bf16)
                    nc.vector.tensor_copy(xt_sb[:], xt_ps[:])

                    h3_ps = ps_h3.tile([D, TILE_ROWS], f32)
                    for half in range(TILE_ROWS // 512):
                        nc.tensor.matmul(
                            h3_ps[:, 512 * half:512 * (half + 1)],
                            wc_sb[:],
                            xt_sb[:, 512 * half:512 * (half + 1)],
                            start=True, stop=True)

                    relu_sb = rpool.tile([D, TILE_ROWS], bf16)
                    a = min(act_cols, TILE_ROWS)
                    if a > 0:
                        nc.scalar.activation(
                            relu_sb[:, 0:a], h3_ps[:, 0:a],
                            mybir.ActivationFunctionType.Relu,
                            bias=bc_sb[:, 0:1], scale=1.0,
                        )
                    if a < TILE_ROWS:
                        nc.vector.tensor_scalar(
                            relu_sb[:, a:TILE_ROWS], h3_ps[:, a:TILE_ROWS],
                            bc_sb[:, 0:1], 0.0,
                            mybir.AluOpType.add, mybir.AluOpType.max,
                        )

                    for k in range(BLOCKS_PER_TILE):
                        g = t * BLOCKS_PER_TILE + k
                        nc.tensor.matmul(
                            pq[:, NQ * g:NQ * (g + 1)],
                            relu_sb[:, 128 * k:128 * (k + 1)],
                            w45_sb[:],
                            start=(g == 0), stop=False,
                        )
                # accumulate biases b45 over the whole chunk's pq
                nc.tensor.matmul(pq[:], ones1_sb[0:1, :], b45t_sb[0:1, :],
                                 start=False, stop=True)

                pq3 = pq[:].rearrange("p (g n) -> p g n", n=NQ)
                e_sb = spool.tile([D, STATS_W], f32, tag="e")
                e3 = e_sb[:].rearrange("p (g n) -> p g n", n=5)
                nc.scalar.activation(e3, pq3[:, :, 5:10],
                                     mybir.ActivationFunctionType.Exp)
                pe_sb = spool.tile([D, STATS_W], f32, tag="pe")
                pe3 = pe_sb[:].rearrange("p (g n) -> p g n", n=5)
                nc.vector.tensor_mul(pe3, pq3[:, :, 0:5], e3)
                if chunk == 0:
                    nc.vector.tensor_copy(acc_e[:], e_sb[:])
                    nc.vector.tensor_copy(acc_pe[:], pe_sb[:])
                else:
                    nc.vector.tensor_add(acc_e[:], acc_e[:], e_sb[:])
                    nc.vector.tensor_add(acc_pe[:], acc_pe[:], pe_sb[:])

            nc.sync.dma_start(out=out_ext[:, 0:STATS_W], in_=acc_e[:])
            nc.sync.dma_start(out=out_ext[:, STATS_W:2 * STATS_W],
                              in_=acc_pe[:])

    nc.compile()
    return nc


def _prep_consts(W1, b1, W2, b2, W3, b3, W4, b4, alpha):
    Wc = (W1.astype(np.float64) @ W2.astype(np.float64)
          @ W3.astype(np.float64))
    bc = ((b1.astype(np.float64) @ W2.astype(np.float64)
           + b2.astype(np.float64)) @ W3.astype(np.float64)
          + b3.astype(np.float64))
    W4a = W4.astype(np.float64) * alpha.astype(np.float64)[None, :]
    b4a = b4.astype(np.float64) * alpha.astype(np.float64)
    W45 = np.concatenate([W4.astype(np.float64), W4a], axis=1)  # [128, 10]
    b45 = np.concatenate([b4.astype(np.float64), b4a])  # [10]
    b45t = np.tile(b45, GROUPS_PER_CHUNK)[None, :]  # [1, 320]
    return (
        Wc.astype(BF16),
        bc.astype(np.float32).reshape(D, 1),
        W45.astype(BF16),
        b45t.astype(BF16),
    )


_CACHE = {}


def _get_nc(rows_per_core):
    key = rows_per_core
    if key not in _CACHE:
        _CACHE[key] = build_kernel(rows_per_core)
    return _CACHE[key]


def make_in_maps(x, W1, b1, W2, b2, W3, b3, W4, b4, alpha):
    x = np.asarray(x)
    n_total = x.shape[1]
    rows_per_core = n_total // N_CORES
    wc_bf, bc_f32, w45_bf, b45t_bf = _prep_consts(
        np.asarray(W1), np.asarray(b1), np.asarray(W2), np.asarray(b2),
        np.asarray(W3), np.asarray(b3), np.asarray(W4), np.asarray(b4),
        np.asarray(alpha))
    # packed bf16 const block: [wc | identity | w45 | ones(row 0)]
    cb = np.zeros((D, 3 * D + NQ), dtype=BF16)
    cb[:, 0:D] = wc_bf
    cb[:, D:2 * D] = np.eye(D, dtype=BF16)
    cb[:, 2 * D:2 * D + NQ] = w45_bf
    cb[0, 2 * D + NQ:3 * D + NQ] = np.ones(D, dtype=BF16)

    xs = np.ascontiguousarray(x.reshape(n_total, D))
    in_maps = []
    for c in range(N_CORES):
        in_maps.append({
            "x": xs[c * rows_per_core:(c + 1) * rows_per_core],
            "cb": cb,
            "bc": bc_f32,
            "b45t": b45t_bf,
        })
    return in_maps, rows_per_core


def run(inputs, trace=False, **run_kwargs):
    """Run the kernel; returns (full_output, BassKernelResults)."""
    in_maps, rows_per_core = make_in_maps(**inputs)
    nc = _get_nc(rows_per_core)
    res = run_bass_kernel_spmd(nc, in_maps, list(range(N_CORES)),
                               trace=trace, **run_kwargs)
    return _finish(res.results), res


def kernel(x, W1, b1, W2, b2, W3, b3, W4, b4, alpha):
    out, _ = run(dict(x=x, W1=W1, b1=b1, W2=W2, b2=b2, W3=W3, b3=b3,
                      W4=W4, b4=b4, alpha=alpha))
    return out


def _finish(results):
    S = np.zeros((D, 2 * STATS_W), dtype=np.float64)
    for r in results:
        S += r["out"].astype(np.float64)
    se = S[:, :STATS_W].reshape(D, GROUPS_PER_CHUNK, 5).sum(axis=(0, 1))
    spe = S[:, STATS_W:].reshape(D, GROUPS_PER_CHUNK, 5).sum(axis=(0, 1))
    return (spe / se)[None, :].astype(np.float32)
